# revision 1
# baseline (speedup 1.0000x reference)
"""Trainium2 Bass kernel for nn_ContactPredictionHead.

Math: reference computes
    logits[b,i,j,o] = sym_{ij}( (h_i*h_j).Wp[o] + (hd_i - hd_j) + bias[o] )
The difference term is antisymmetric in (i,j), so the symmetrization
cancels it exactly. The output reduces to a weighted gram matrix:
    out[b,i,j,o] = sum_d h[b,i,d] * h[b,j,d] * Wp[o,d] + bias[o]
with Wp = W[:, :D].

Sharding: B=4 batches x O=2 output channels = 8 independent [L,L] gram
matrices -> one per NeuronCore. Each core computes
    C = (hT * w).T @ hT   (contraction over D=1280)
where hT = h[b].T is provided pre-transposed by the host so both matmul
operands have the contraction dim on SBUF partitions with contiguous DMA.

C is symmetric, so only upper-triangle blocks are computed on the PE
(~55% of the FLOPs); the strictly-lower blocks are produced by PE-mode
transposes of the staged upper blocks and mirrored to DRAM.

Performance notes (per core, HW-calibrated against TimelineSim):
- matmuls run in float32r (full PE rate at moving-dim >= 256, vs 4x
  slower for exact fp32); measured L2 rel err vs fp64 = 2.1e-4.
- full-grid f32r baseline: 164 us -> triangle+mirror, split DMA rings
  (inputs on SP, outputs on ACT), jc-major load order, ACT-drained
  mirror copies, last-wave outputs rerouted to the idle SP ring so
  the ACT sequencer is free for the tail mirror copies (-0.5 us):
  106.4 us one-shot model; 129.5 us measured for the
  full load+compute+store pipeline in hardware-loop mode (upper bound:
  includes per-pass input reload WAR serialization).
- PE busy ~90 us of which ~75 us is matmul streaming at 2.4 GHz and
  ~12 us mirror transposes; DVE ~47 us; DMA ~76 us. Compute-bound.
- A/B results kept as flags: WIDE_DIAG=True (400x512-wide MMs instead
  of 560 narrower ones) measured 141 us - LDWEIGHTS is well hidden for
  f32r, so narrow diagonal groups win; GADGET (defer output DMAs past
  the input load) measured neutral; stage bufs 4 beats 6;
  PSUM_SHARED=True (transposes rotate through the matmul psum slots)
  modeled 136 us vs 107 - a dedicated transpose bank wins.
"""

import numpy as np

B, L, D, O = 4, 2048, 1280, 2
P = 128
DT = D // P          # 10 contraction tiles of 128
NT = 512             # psum bank width (fp32)
MT = L // P          # 16 output row tiles
NTILES = L // NT     # 4 output col tiles

# Matmul input dtype: "f32r" (full-rate, reduced internal precision) or
# "f32" (exact fp32, 4x slower on the PE array).
MM_DTYPE = "f32r"
SYMM = True          # exploit symmetry (triangle + mirror)
MIRROR = "full"      # "full" | "nodma" (transpose but skip mirror DMA) | "none"
GADGET = False       # hold output DMAs behind the input load (measured neutral)
WIDE_DIAG = False    # 512-wide diagonal groups (fewer, wider matmuls)
PSUM_SHARED = False  # transposes allocate from the main 8-bank psum pool

# Benchmark knob: repeat the whole compute R times inside one NEFF so HW
# exec time can be extracted from wall-clock deltas (transfers constant).
REPS = 1

TRACE = False        # test.py sets True to capture an NTFF profile
LAST_RESULT = None   # BassKernelResults of the most recent run (for test.py)

_nc_cache = {}


def _triangle_layout():
    """Upper-triangle matmul groups and the direct-coverage block set.

    Returns (groups, direct) where groups is a list of (m, start, w):
    row-tile m computes output columns [start, start+w). The diagonal
    group is shrunk to the 128-multiple width >= 256 covering the
    diagonal; later column chunks are full 512 wide. direct holds all
    (row_tile, col_block) pairs written by these groups.
    """
    groups = []
    direct = set()
    for m in range(MT):
        n0 = m // 4
        r = m % 4
        if WIDE_DIAG:
            soff, w = 0, 512
        else:
            soff, w = [(0, 512), (128, 384), (256, 256), (256, 256)][r]
        start = 512 * n0 + soff
        chunk_list = [(start, w)] + [(512 * n, 512) for n in range(n0 + 1, NTILES)]
        for s, ww in chunk_list:
            groups.append((m, s, ww))
            for cb in range(s // 128, (s + ww) // 128):
                direct.add((m, cb))
    return groups, direct


def _build_nc():
    key = (MM_DTYPE, SYMM, REPS, MIRROR, GADGET, WIDE_DIAG, PSUM_SHARED)
    if key in _nc_cache:
        return _nc_cache[key]

    import concourse.bass as bass
    import concourse.mybir as mybir
    import concourse.tile as tile
    from concourse import bacc
    from concourse.masks import make_identity

    f32 = mybir.dt.float32
    mm_dt = mybir.dt.float32r if MM_DTYPE == "f32r" else mybir.dt.float32

    nc = bacc.Bacc("TRN2", target_bir_lowering=False, debug=False, num_devices=8)
    ht_dram = nc.dram_tensor("ht", [D, L], mm_dt, kind="ExternalInput")
    w_dram = nc.dram_tensor("wcol", [P, DT], mm_dt, kind="ExternalInput")
    b_dram = nc.dram_tensor("bias", [P, 1], f32, kind="ExternalInput")
    out_dram = nc.dram_tensor("out", [L, L], f32, kind="ExternalOutput")

    ht3 = ht_dram[:, :].rearrange("(t p) l -> p t l", p=P)  # [128, 10, 2048]

    with tile.TileContext(nc) as tc:
        with (
            tc.tile_pool(name="data", bufs=1) as data,
            tc.tile_pool(
                name="psum", bufs=8 if PSUM_SHARED else 7, space="PSUM"
            ) as psum,
            tc.tile_pool(name="psumt", bufs=1, space="PSUM") as _psumt,
            tc.tile_pool(name="stage", bufs=4) as stage,
            tc.tile_pool(name="stage2", bufs=4) as stage2,
        ):
            h_sb = data.tile([P, DT, L], mm_dt)  # hT resident: 80KB/partition
            a_sb = data.tile([P, DT, L], mm_dt)  # scaled copy:  80KB/partition
            w_sb = data.tile([P, DT], mm_dt)
            b_sb = data.tile([P, 1], f32)
            ident = data.tile([P, P], f32)

            make_identity(nc, ident[:, :])
            nc.sync.dma_start(w_sb[:, :], w_dram[:, :])
            nc.sync.dma_start(b_sb[:, :], b_dram[:, :])

            # Load hT in (j-chunk, t) pieces and scale by w broadcast along j.
            # jc-major order: after the first column chunk lands, the first
            # wave of output tiles is fully computable while later chunks
            # stream in, so the PE ramps with the DMA instead of after it.
            def emit_load():
                for jc in range(NTILES):
                    for t in range(DT):
                        js = bass.ts(jc, NT)
                        nc.sync.dma_start(h_sb[:, t, js], ht3[:, t, js])
                        nc.vector.tensor_tensor(
                            a_sb[:, t, js],
                            h_sb[:, t, js],
                            w_sb[:, t, None].to_broadcast((P, NT)),
                            mybir.AluOpType.mult,
                        )

            if SYMM:
                groups, direct = _triangle_layout()
            else:
                groups = [(m, 512 * n, 512) for m in range(MT) for n in range(NTILES)]
                direct = None

            # Wavefront order matching DMA chunk availability. Within a
            # wave, mirror-heavy groups first so the transpose+copy+DMA
            # mirror pipeline of the final wave drains behind the last
            # matmuls instead of extending the tail.
            def ready_chunk(g):
                m, s, w = g
                return max(m // 4, (s + w - 1) // 512)

            def n_mirrors(g):
                if direct is None:
                    return 0
                m, s, w = g
                return sum(
                    1
                    for cb in range(s // 128, (s + w) // 128)
                    if cb > m and (cb, m) not in direct
                )

            groups = sorted(
                groups, key=lambda g: (ready_chunk(g), -n_mirrors(g), g[0], g[1])
            )

            # Hold the ACT-ring (all output/mirror DMAs) behind the input
            # load: engine sequencers issue DMAs in order, so one dummy
            # ACT-ring DMA that reads the last input chunk keeps output
            # traffic off the HBM while input chunks stream in at full
            # bandwidth (the PE ramp is gated by input chunk arrival).
            scrap = data.tile([P, 1], mm_dt)

            def emit_gadget():
                if GADGET:
                    nc.scalar.dma_start(scrap[:, :], ht3[:, DT - 1, L - 1 : L])

            def emit_mirror(m, s, w, st):
                # Mirror strictly-lower blocks: out[cb*128.., m*128..] =
                # T(st[:, cb-block]) for covered col-blocks cb > m not
                # already written directly by row cb's diagonal group.
                cbs = [
                    cb
                    for cb in range(s // 128, (s + w) // 128)
                    if cb > m and (cb, m) not in direct
                ]
                if not cbs:
                    return
                nmir = len(cbs)
                st2 = stage2.tile([P, NT], f32, name="st2")[:, : nmir * P]
                if PSUM_SHARED:
                    pt = psum.tile([P, NT], f32, name="ps")[:, : nmir * P]
                else:
                    pt = _psumt.tile([P, NT], f32, name="pt")[:, : nmir * P]
                for i, cb in enumerate(cbs):
                    nc.tensor.transpose(
                        pt[:, bass.ts(i, P)],
                        st[:, bass.ds(cb * P - s, P)],
                        ident[:, :],
                    )
                # drain the transposed PSUM on the (otherwise idle) ACT
                # engine so the DVE keeps up with the main bias-add drains
                nc.scalar.activation(st2, pt, mybir.ActivationFunctionType.Copy)
                if MIRROR == "nodma":
                    return
                # one DMA: consecutive row-tiles cbs[0]..cbs[-1], col m
                dst = out_dram[
                    bass.ds(cbs[0] * P, nmir * P), bass.ts(m, P)
                ].rearrange("(t p) c -> p t c", p=P)
                nc.scalar.dma_start(dst, st2.rearrange("p (t c) -> p t c", c=P))

            def emit_groups():
                # NOTE: emitting mirror work one group late (to give a
                # psumt-stalled transpose slack before queued matmuls) was
                # modeled at 107.3 us vs 107.1 inline - no benefit, so
                # mirrors stay inline with their producing group.
                for m, s, w in groups:
                    ps = psum.tile([P, NT], f32, name="ps")[:, :w]
                    for k in range(DT):
                        nc.tensor.matmul(
                            ps,
                            a_sb[:, k, bass.ts(m, P)],
                            h_sb[:, k, bass.ds(s, w)],
                            start=(k == 0),
                            stop=(k == DT - 1),
                        )
                    st = stage.tile([P, NT], f32, name="st")[:, :w]
                    # copy PSUM->SBUF fused with the (per-partition) bias add
                    nc.vector.tensor_tensor(
                        st,
                        ps,
                        b_sb[:, 0, None].to_broadcast((P, w)),
                        mybir.AluOpType.add,
                    )
                    # outputs go out on the ACT HWDGE ring so they never
                    # queue behind the input loads on the SP ring (FIFO per
                    # issuing engine on HW). Last-wave outputs use the SP
                    # ring instead (loads are done by then), so the ACT
                    # sequencer isn't dispatching DMAs when the final
                    # mirror copies need it.
                    out_eng = nc.sync if ready_chunk((m, s, w)) == 3 else nc.scalar
                    out_eng.dma_start(out_dram[bass.ts(m, P), bass.ds(s, w)], st)

                    if SYMM and MIRROR != "none":
                        emit_mirror(m, s, w, st)

            if REPS == 1:
                emit_load()
                emit_gadget()
                emit_groups()
            else:
                # benchmark-only hardware loop (same compile size, R passes).
                # The load sits inside the loop so a pass measures the full
                # pipeline including the input-load overlap.
                with tc.For_i(0, REPS, 1):
                    emit_load()
                    emit_gadget()
                    emit_groups()

    nc.compile()
    _nc_cache[key] = nc
    return nc


def kernel(hidden_states, W, b):
    global LAST_RESULT
    from concourse.bass_utils import run_bass_kernel_spmd

    hidden_states = np.asarray(hidden_states, dtype=np.float32)
    W = np.asarray(W, dtype=np.float32)
    b = np.asarray(b, dtype=np.float32)

    Wp = W[:, :D]                                   # [O, D]
    # hT per batch, contiguous [D, L]
    hT = np.ascontiguousarray(hidden_states.transpose(0, 2, 1))

    in_maps = []
    for c in range(8):
        bb, o = divmod(c, 2)
        wcol = np.ascontiguousarray(Wp[o].reshape(DT, P).T)  # [P, DT], w[t*128+p]
        bias = np.full((P, 1), b[o], dtype=np.float32)
        in_maps.append({"ht": hT[bb], "wcol": wcol, "bias": bias})

    nc = _build_nc()
    res = run_bass_kernel_spmd(nc, in_maps, core_ids=list(range(8)), trace=TRACE)
    LAST_RESULT = res

    out = np.empty((B, L, L, O), dtype=np.float32)
    for c in range(8):
        bb, o = divmod(c, 2)
        out[bb, :, :, o] = res.results[c]["out"]
    return out

